# revision 25
# baseline (speedup 1.0000x reference)
"""Trainium2 Bass kernel for nn_BlockConvolutionLean.

Computation (see reference):
  features = einsum('nse,te->nst', seq_vector, W)        # 1x1 conv
  blocks of BS=8 along S; out = exclusive-cumsum within block + b_eff
  b_eff = bias with bias[0] doubled at position 0.

Key identity: per 8-token block, Out = L @ X @ W^T + b_eff where L is the
strictly-lower-triangular 8x8 ones matrix.  Both factors are matmuls:

  mmA (cumsum fused into transpose):  Z[e, s'] = sum_s X[s, e] * LT[s, s']
      where LT[s, s'] = 1 iff s, s' in same 8-block and s < s'
      (LT is the 128x128 block-diagonal strictly-upper ones matrix).
      lhsT = X chunk [s(128 part), e(128)] -- natural DMA layout!
      rhs  = LT [s(128 part), s'(128)]
      => Z = (L@X)^T lands e-on-partitions, exactly what mmB needs.

  mmB (projection): O[s', t] = sum_e Z[e, s'] * WT[e, t], K=E=256 via two
      accumulating matmuls.  O is token-on-partition => contiguous store.

  bias: per-partition bias add fused into the PSUM->SBUF eviction
      (ScalarE activation Identity with bias AP [128,1], b_eff[p % 8]).

Sharding: data-parallel over N=8 batches, one batch per NeuronCore.

Precision modes (BCONV_MODE):
  f32   : everything float32 (PE-bound: fp32 matmul = 4 cyc/row)
  f32r  : float32r matmuls (single-pass fp32; LDWEIGHTS-bound)
  bf16  : host stages x as bf16 (halves input HBM traffic); all matmul
          operands bf16, fp32 PSUM accumulation, f32 output
  f16   : like bf16 but float16 operands (10-bit mantissa) AND the
          output staged as f16 (host upcasts) -- halves store traffic
"""

import os

import numpy as np

import concourse.mybir as mybir
import concourse.tile as tile
from concourse import bacc
from concourse.bass_utils import run_bass_kernel_spmd

N, S, E, BS = 8, 8192, 256, 8
P = 128                 # tokens per tile / partitions
NCORES = 8
G = 4                   # j-tiles per PSUM group (two 2KB banks each)
# chunk schedule in 128-token tiles: small chunks at both ends cut the
# time-to-first-matmul and the last-store tail; 8-tile steady state
SCHED = [4, 4] + [8] * 6 + [4, 4]
assert sum(SCHED) == S // P and all(c % G == 0 for c in SCHED)

MODE = os.environ.get("BCONV_MODE", "f16")

_cache = {}


def _build_nc(mode: str):
    f32 = mybir.dt.float32
    f32r = mybir.dt.float32r
    bf16 = mybir.dt.bfloat16
    f16 = mybir.dt.float16

    # a_dt: mmA operands (x, lt); b_dt: mmB operands (z, wt);
    # o_dt: output staging dtype
    if mode == "f32":
        a_dt, b_dt, o_dt = f32, f32, f32
    elif mode == "f32r":
        a_dt, b_dt, o_dt = f32r, f32r, f32
    elif mode == "bf16":
        a_dt, b_dt, o_dt = bf16, bf16, f32
    elif mode == "f16":
        a_dt, b_dt, o_dt = f16, f16, f16
    else:
        raise ValueError(mode)

    nc = bacc.Bacc(
        "TRN2", target_bir_lowering=False, debug=False, num_devices=NCORES)
    # x arrives pre-staged in the matmul dtype (host casts for bf16;
    # f32r is bit-identical to f32) -> every load is a plain HWDGE DMA.
    x = nc.dram_tensor("x", [S, E], a_dt, kind="ExternalInput")
    # lt and wt fused into one tensor -> one DMA, one completion wait
    cw = nc.dram_tensor("cw", [P, P + 2 * E], a_dt, kind="ExternalInput")
    beff = nc.dram_tensor("beff", [P, 1], f32, kind="ExternalInput")
    y = nc.dram_tensor("y", [S, E], o_dt, kind="ExternalOutput")

    ident = mybir.ActivationFunctionType.Identity
    max_pack = max(SCHED)

    with tile.TileContext(nc) as tc:
        with (
            tc.tile_pool(name="const", bufs=1) as constp,
            tc.tile_pool(name="xin", bufs=6) as xin,
            tc.tile_pool(name="zsb", bufs=6) as zsbp,
            tc.tile_pool(name="yout", bufs=8) as yout,
            tc.tile_pool(name="zps", bufs=2, space="PSUM") as zps,
            tc.tile_pool(name="ops", bufs=2, space="PSUM") as ops,
        ):
            cw_sb = constp.tile([P, P + 2 * E], a_dt)
            nc.sync.dma_start(cw_sb[:], cw[:, :])
            lt_sb = cw_sb[:, 0:P]
            wt_sb = cw_sb[:, P:P + 2 * E]
            beff_sb = constp.tile([P, 1], f32)

            # HAM warmup: keep the PE busy while the first input chunks
            # stream in, so real matmuls start at 2.4 GHz (K=8/8).
            wset = constp.tile([P, P], a_dt)
            nc.gpsimd.memset(wset[:], 0.0)
            wp = ops.tile([P, P], f32, tag="op")
            for _ in range(28):
                nc.tensor.matmul(wp[:], wset[:], wset[:], start=True, stop=True)

            tile0 = 0
            for ci, pack in enumerate(SCHED):
                chunk = pack * P
                xsl = x[tile0 * P:tile0 * P + chunk, :].rearrange(
                    "(i p) e -> p i e", p=P)
                xb = xin.tile([P, max_pack * E], a_dt, tag="xb")
                nc.sync.dma_start(xb[:, 0:pack * E], xsl)
                if ci == 0:
                    # bias needed only at the first eviction
                    nc.sync.dma_start(beff_sb[:], beff[:, :])

                for h in range(pack // G):
                    # mmA group: 2*G tiles of (L@X)^T into one PSUM bank
                    zp = zps.tile([P, 2 * G * P], f32)
                    for jj in range(G):
                        j = G * h + jj
                        for k in range(2):
                            m = 2 * jj + k
                            nc.tensor.matmul(
                                zp[:, m * P:(m + 1) * P],
                                xb[:, j * E + k * P: j * E + (k + 1) * P],
                                lt_sb,
                                start=True, stop=True,
                            )
                    zt = zsbp.tile([P, 2 * G * P], b_dt)
                    nc.vector.tensor_copy(zt[:], zp[:])
                    # mmB group: G projected j-tiles into one PSUM bank
                    op = ops.tile([P, G * E], f32)
                    for jj in range(G):
                        nc.tensor.matmul(
                            op[:, jj * E:(jj + 1) * E],
                            zt[:, 2 * jj * P:(2 * jj + 1) * P],
                            wt_sb[:, 0:E],
                            start=True, stop=False)
                        nc.tensor.matmul(
                            op[:, jj * E:(jj + 1) * E],
                            zt[:, (2 * jj + 1) * P:(2 * jj + 2) * P],
                            wt_sb[:, E:2 * E],
                            start=False, stop=True)
                    # eviction with fused per-partition bias add
                    ot = yout.tile([P, G * E], o_dt)
                    nc.scalar.activation(ot[:], op[:], ident, bias=beff_sb[:])
                    ysl = y[(tile0 + h * G) * P:(tile0 + (h + 1) * G) * P,
                            :].rearrange("(i p) e -> p i e", p=P)
                    # stores ride SWDGE so input prefetch never queues
                    # behind them on the HWDGE FIFO; the last chunks go
                    # back on HWDGE (inputs are done) to shorten the
                    # SWDGE drain in the kernel tail
                    if ci >= len(SCHED) - 4:
                        nc.sync.dma_start(ysl, ot[:])
                    else:
                        nc.gpsimd.dma_start(ysl, ot[:])
                tile0 += pack
    nc.compile()
    return nc


def _np_dt(name):
    import ml_dtypes
    return {"bfloat16": ml_dtypes.bfloat16}.get(name, np.float32)


def _host_consts(W: np.ndarray, b: np.ndarray, mode: str):
    if mode in ("f32", "f32r"):
        a_np, b_np = np.float32, np.float32
    elif mode == "bf16":
        a_np, b_np = _np_dt("bfloat16"), _np_dt("bfloat16")
    elif mode == "f16":
        a_np, b_np = np.float16, np.float16
    else:
        raise ValueError(mode)

    idx = np.arange(P)
    blk = idx // BS
    LT = ((blk[:, None] == blk[None, :]) & (idx[:, None] < idx[None, :]))
    LT = LT.astype(a_np)

    WT = W.T.astype(np.float32)                      # [E, T]
    WTP = np.concatenate([WT[0:P, :], WT[P:2 * P, :]], axis=1)  # [128, 512]
    WTP = WTP.astype(b_np)
    CW = np.ascontiguousarray(np.concatenate([LT, WTP], axis=1))

    be = b.astype(np.float64).copy()
    be[0] += be[0]
    BEFF = np.ascontiguousarray(
        be[idx % BS].astype(np.float32).reshape(P, 1))
    return CW, BEFF, a_np


def kernel(seq_vector, W, bias):
    mode = MODE
    x = np.asarray(seq_vector, dtype=np.float32)
    W = np.asarray(W, dtype=np.float32)
    b = np.asarray(bias, dtype=np.float32)
    assert x.shape == (N, S, E)

    CW, BEFF, a_np = _host_consts(W, b, mode)
    xs = np.ascontiguousarray(x.astype(a_np))

    if mode not in _cache:
        _cache[mode] = _build_nc(mode)
    nc = _cache[mode]

    in_maps = [
        {"x": xs[i], "cw": CW, "beff": BEFF}
        for i in range(NCORES)
    ]
    res = run_bass_kernel_spmd(nc, in_maps, core_ids=list(range(NCORES)))
    out = np.stack(
        [np.asarray(r["y"], dtype=np.float32) for r in res.results], axis=0)
    return out.reshape(N, S, E)


# revision 26
# speedup vs baseline: 1.0862x; 1.0862x over previous
"""Trainium2 Bass kernel for nn_BlockConvolutionLean.

Computation (see reference):
  features = einsum('nse,te->nst', seq_vector, W)        # 1x1 conv
  blocks of BS=8 along S; out = exclusive-cumsum within block + b_eff
  b_eff = bias with bias[0] doubled at position 0.

Key identity: per 8-token block, Out = L @ X @ W^T + b_eff where L is the
strictly-lower-triangular 8x8 ones matrix.  Both factors are matmuls:

  mmA (cumsum fused into transpose):  Z[e, s'] = sum_s X[s, e] * LT[s, s']
      where LT[s, s'] = 1 iff s, s' in same 8-block and s < s'
      (LT is the 128x128 block-diagonal strictly-upper ones matrix).
      lhsT = X chunk [s(128 part), e(128)] -- natural DMA layout!
      rhs  = LT [s(128 part), s'(128)]
      => Z = (L@X)^T lands e-on-partitions, exactly what mmB needs.

  mmB (projection): O[s', t] = sum_e Z[e, s'] * WT[e, t], K=E=256 via two
      accumulating matmuls.  O is token-on-partition => contiguous store.

  bias: per-partition bias add fused into the PSUM->SBUF eviction
      (ScalarE activation Identity with bias AP [128,1], b_eff[p % 8]).

Sharding: data-parallel over N=8 batches, one batch per NeuronCore.

Precision modes (BCONV_MODE):
  f32   : everything float32 (PE-bound: fp32 matmul = 4 cyc/row)
  f32r  : float32r matmuls (single-pass fp32; LDWEIGHTS-bound)
  bf16  : host stages x as bf16 (halves input HBM traffic); all matmul
          operands bf16, fp32 PSUM accumulation, f32 output
  f16   : like bf16 but float16 operands (10-bit mantissa) AND the
          output staged as f16 (host upcasts) -- halves store traffic
"""

import os

import numpy as np

import concourse.mybir as mybir
import concourse.tile as tile
from concourse import bacc
from concourse.bass_utils import run_bass_kernel_spmd

N, S, E, BS = 8, 8192, 256, 8
P = 128                 # tokens per tile / partitions
NCORES = 8
G = 2                   # j-tiles per PSUM group (one 2KB bank each)
# chunk schedule in 128-token tiles: small chunks at both ends cut the
# time-to-first-matmul and the last-store tail; 8-tile steady state
SCHED = [2, 2, 4] + [8] * 6 + [4, 2, 2]
assert sum(SCHED) == S // P and all(c % G == 0 for c in SCHED)

MODE = os.environ.get("BCONV_MODE", "f16")

_cache = {}


def _build_nc(mode: str):
    f32 = mybir.dt.float32
    f32r = mybir.dt.float32r
    bf16 = mybir.dt.bfloat16
    f16 = mybir.dt.float16

    # a_dt: mmA operands (x, lt); b_dt: mmB operands (z, wt);
    # o_dt: output staging dtype
    if mode == "f32":
        a_dt, b_dt, o_dt = f32, f32, f32
    elif mode == "f32r":
        a_dt, b_dt, o_dt = f32r, f32r, f32
    elif mode == "bf16":
        a_dt, b_dt, o_dt = bf16, bf16, f32
    elif mode == "f16":
        a_dt, b_dt, o_dt = f16, f16, f16
    else:
        raise ValueError(mode)

    nc = bacc.Bacc(
        "TRN2", target_bir_lowering=False, debug=False, num_devices=NCORES)
    # x arrives pre-staged in the matmul dtype (host casts for bf16;
    # f32r is bit-identical to f32) -> every load is a plain HWDGE DMA.
    x = nc.dram_tensor("x", [S, E], a_dt, kind="ExternalInput")
    # lt and wt fused into one tensor -> one DMA, one completion wait
    cw = nc.dram_tensor("cw", [P, P + 2 * E], a_dt, kind="ExternalInput")
    beff = nc.dram_tensor("beff", [P, 1], f32, kind="ExternalInput")
    y = nc.dram_tensor("y", [S, E], o_dt, kind="ExternalOutput")

    ident = mybir.ActivationFunctionType.Identity
    max_pack = max(SCHED)

    with tile.TileContext(nc) as tc:
        with (
            tc.tile_pool(name="const", bufs=1) as constp,
            tc.tile_pool(name="xin", bufs=6) as xin,
            tc.tile_pool(name="zsb", bufs=6) as zsbp,
            tc.tile_pool(name="yout", bufs=8) as yout,
            tc.tile_pool(name="zps", bufs=3, space="PSUM") as zps,
            tc.tile_pool(name="ops", bufs=3, space="PSUM") as ops,
            tc.tile_pool(name="wps", bufs=1, space="PSUM") as wps,
        ):
            cw_sb = constp.tile([P, P + 2 * E], a_dt)
            nc.sync.dma_start(cw_sb[:], cw[:, :])
            lt_sb = cw_sb[:, 0:P]
            wt_sb = cw_sb[:, P:P + 2 * E]
            beff_sb = constp.tile([P, 1], f32)

            # HAM warmup: keep the PE busy while the first input chunks
            # stream in, so real matmuls start at 2.4 GHz (K=8/8).
            wset = constp.tile([P, P], a_dt)
            nc.gpsimd.memset(wset[:], 0.0)
            wp = wps.tile([P, P], f32)
            for _ in range(28):
                nc.tensor.matmul(wp[:], wset[:], wset[:], start=True, stop=True)

            tile0 = 0
            for ci, pack in enumerate(SCHED):
                chunk = pack * P
                xsl = x[tile0 * P:tile0 * P + chunk, :].rearrange(
                    "(i p) e -> p i e", p=P)
                xb = xin.tile([P, max_pack * E], a_dt, tag="xb")
                nc.sync.dma_start(xb[:, 0:pack * E], xsl)
                if ci == 0:
                    # bias needed only at the first eviction
                    nc.sync.dma_start(beff_sb[:], beff[:, :])

                for h in range(pack // G):
                    # mmA group: 2*G tiles of (L@X)^T into one PSUM bank
                    zp = zps.tile([P, 2 * G * P], f32)
                    for jj in range(G):
                        j = G * h + jj
                        for k in range(2):
                            m = 2 * jj + k
                            nc.tensor.matmul(
                                zp[:, m * P:(m + 1) * P],
                                xb[:, j * E + k * P: j * E + (k + 1) * P],
                                lt_sb,
                                start=True, stop=True,
                            )
                    zt = zsbp.tile([P, 2 * G * P], b_dt)
                    nc.vector.tensor_copy(zt[:], zp[:])
                    # mmB group: G projected j-tiles into one PSUM bank
                    op = ops.tile([P, G * E], f32)
                    for jj in range(G):
                        nc.tensor.matmul(
                            op[:, jj * E:(jj + 1) * E],
                            zt[:, 2 * jj * P:(2 * jj + 1) * P],
                            wt_sb[:, 0:E],
                            start=True, stop=False)
                        nc.tensor.matmul(
                            op[:, jj * E:(jj + 1) * E],
                            zt[:, (2 * jj + 1) * P:(2 * jj + 2) * P],
                            wt_sb[:, E:2 * E],
                            start=False, stop=True)
                    # eviction with fused per-partition bias add
                    ot = yout.tile([P, G * E], o_dt)
                    nc.scalar.activation(ot[:], op[:], ident, bias=beff_sb[:])
                    ysl = y[(tile0 + h * G) * P:(tile0 + (h + 1) * G) * P,
                            :].rearrange("(i p) e -> p i e", p=P)
                    # stores ride SWDGE so input prefetch never queues
                    # behind them on the HWDGE FIFO; the last chunks go
                    # back on HWDGE (inputs are done) to shorten the
                    # SWDGE drain in the kernel tail
                    if ci >= len(SCHED) - 4:
                        nc.sync.dma_start(ysl, ot[:])
                    else:
                        nc.gpsimd.dma_start(ysl, ot[:])
                tile0 += pack
    nc.compile()
    return nc


def _np_dt(name):
    import ml_dtypes
    return {"bfloat16": ml_dtypes.bfloat16}.get(name, np.float32)


def _host_consts(W: np.ndarray, b: np.ndarray, mode: str):
    if mode in ("f32", "f32r"):
        a_np, b_np = np.float32, np.float32
    elif mode == "bf16":
        a_np, b_np = _np_dt("bfloat16"), _np_dt("bfloat16")
    elif mode == "f16":
        a_np, b_np = np.float16, np.float16
    else:
        raise ValueError(mode)

    idx = np.arange(P)
    blk = idx // BS
    LT = ((blk[:, None] == blk[None, :]) & (idx[:, None] < idx[None, :]))
    LT = LT.astype(a_np)

    WT = W.T.astype(np.float32)                      # [E, T]
    WTP = np.concatenate([WT[0:P, :], WT[P:2 * P, :]], axis=1)  # [128, 512]
    WTP = WTP.astype(b_np)
    CW = np.ascontiguousarray(np.concatenate([LT, WTP], axis=1))

    be = b.astype(np.float64).copy()
    be[0] += be[0]
    BEFF = np.ascontiguousarray(
        be[idx % BS].astype(np.float32).reshape(P, 1))
    return CW, BEFF, a_np


def kernel(seq_vector, W, bias):
    mode = MODE
    x = np.asarray(seq_vector, dtype=np.float32)
    W = np.asarray(W, dtype=np.float32)
    b = np.asarray(bias, dtype=np.float32)
    assert x.shape == (N, S, E)

    CW, BEFF, a_np = _host_consts(W, b, mode)
    xs = np.ascontiguousarray(x.astype(a_np))

    if mode not in _cache:
        _cache[mode] = _build_nc(mode)
    nc = _cache[mode]

    in_maps = [
        {"x": xs[i], "cw": CW, "beff": BEFF}
        for i in range(NCORES)
    ]
    res = run_bass_kernel_spmd(nc, in_maps, core_ids=list(range(NCORES)))
    out = np.stack(
        [np.asarray(r["y"], dtype=np.float32) for r in res.results], axis=0)
    return out.reshape(N, S, E)


# revision 27
# speedup vs baseline: 1.0889x; 1.0025x over previous
"""Trainium2 Bass kernel for nn_BlockConvolutionLean.

Computation (see reference):
  features = einsum('nse,te->nst', seq_vector, W)        # 1x1 conv
  blocks of BS=8 along S; out = exclusive-cumsum within block + b_eff
  b_eff = bias with bias[0] doubled at position 0.

Key identity: per 8-token block, Out = L @ X @ W^T + b_eff where L is the
strictly-lower-triangular 8x8 ones matrix.  Both factors are matmuls:

  mmA (cumsum fused into transpose):  Z[e, s'] = sum_s X[s, e] * LT[s, s']
      where LT[s, s'] = 1 iff s, s' in same 8-block and s < s'
      (LT is the 128x128 block-diagonal strictly-upper ones matrix).
      lhsT = X chunk [s(128 part), e(128)] -- natural DMA layout!
      rhs  = LT [s(128 part), s'(128)]
      => Z = (L@X)^T lands e-on-partitions, exactly what mmB needs.

  mmB (projection): O[s', t] = sum_e Z[e, s'] * WT[e, t], K=E=256 via two
      accumulating matmuls.  O is token-on-partition => contiguous store.

  bias: per-partition bias add fused into the PSUM->SBUF eviction
      (ScalarE activation Identity with bias AP [128,1], b_eff[p % 8]).

Sharding: data-parallel over N=8 batches, one batch per NeuronCore.

Precision modes (BCONV_MODE):
  f32   : everything float32 (PE-bound: fp32 matmul = 4 cyc/row)
  f32r  : float32r matmuls (single-pass fp32; LDWEIGHTS-bound)
  bf16  : host stages x as bf16 (halves input HBM traffic); all matmul
          operands bf16, fp32 PSUM accumulation, f32 output
  f16   : like bf16 but float16 operands (10-bit mantissa) AND the
          output staged as f16 (host upcasts) -- halves store traffic
"""

import os

import numpy as np

import concourse.mybir as mybir
import concourse.tile as tile
from concourse import bacc
from concourse.bass_utils import run_bass_kernel_spmd

N, S, E, BS = 8, 8192, 256, 8
P = 128                 # tokens per tile / partitions
NCORES = 8
G = 2                   # j-tiles per PSUM group (one 2KB bank each)
# chunk schedule in 128-token tiles: small chunks at both ends cut the
# time-to-first-matmul and the last-store tail; 8-tile steady state
SCHED = [2, 2, 4] + [8] * 6 + [4, 2, 2]
assert sum(SCHED) == S // P and all(c % G == 0 for c in SCHED)

MODE = os.environ.get("BCONV_MODE", "f16")

_cache = {}


def _build_nc(mode: str):
    f32 = mybir.dt.float32
    f32r = mybir.dt.float32r
    bf16 = mybir.dt.bfloat16
    f16 = mybir.dt.float16

    # a_dt: mmA operands (x, lt); b_dt: mmB operands (z, wt);
    # o_dt: output staging dtype
    if mode == "f32":
        a_dt, b_dt, o_dt = f32, f32, f32
    elif mode == "f32r":
        a_dt, b_dt, o_dt = f32r, f32r, f32
    elif mode == "bf16":
        a_dt, b_dt, o_dt = bf16, bf16, f32
    elif mode == "f16":
        a_dt, b_dt, o_dt = f16, f16, f16
    else:
        raise ValueError(mode)

    nc = bacc.Bacc(
        "TRN2", target_bir_lowering=False, debug=False, num_devices=NCORES)
    # x arrives pre-staged in the matmul dtype (host casts for bf16;
    # f32r is bit-identical to f32) -> every load is a plain HWDGE DMA.
    x = nc.dram_tensor("x", [S, E], a_dt, kind="ExternalInput")
    # lt and wt fused into one tensor -> one DMA, one completion wait
    cw = nc.dram_tensor("cw", [P, P + 2 * E], a_dt, kind="ExternalInput")
    beff = nc.dram_tensor("beff", [P, 1], f32, kind="ExternalInput")
    y = nc.dram_tensor("y", [S, E], o_dt, kind="ExternalOutput")

    ident = mybir.ActivationFunctionType.Identity
    max_pack = max(SCHED)

    with tile.TileContext(nc) as tc:
        with (
            tc.tile_pool(name="const", bufs=1) as constp,
            tc.tile_pool(name="xin", bufs=8) as xin,
            tc.tile_pool(name="zsb", bufs=8) as zsbp,
            tc.tile_pool(name="yout", bufs=8) as yout,
            tc.tile_pool(name="zps", bufs=3, space="PSUM") as zps,
            tc.tile_pool(name="ops", bufs=3, space="PSUM") as ops,
            tc.tile_pool(name="wps", bufs=1, space="PSUM") as wps,
        ):
            cw_sb = constp.tile([P, P + 2 * E], a_dt)
            nc.sync.dma_start(cw_sb[:], cw[:, :])
            lt_sb = cw_sb[:, 0:P]
            wt_sb = cw_sb[:, P:P + 2 * E]
            beff_sb = constp.tile([P, 1], f32)

            # HAM warmup: keep the PE busy while the first input chunks
            # stream in, so real matmuls start at 2.4 GHz (K=8/8).
            wset = constp.tile([P, P], a_dt)
            nc.gpsimd.memset(wset[:], 0.0)
            wp = wps.tile([P, P], f32)
            for _ in range(28):
                nc.tensor.matmul(wp[:], wset[:], wset[:], start=True, stop=True)

            tile0 = 0
            for ci, pack in enumerate(SCHED):
                chunk = pack * P
                xsl = x[tile0 * P:tile0 * P + chunk, :].rearrange(
                    "(i p) e -> p i e", p=P)
                xb = xin.tile([P, max_pack * E], a_dt, tag="xb")
                nc.sync.dma_start(xb[:, 0:pack * E], xsl)
                if ci == 0:
                    # bias needed only at the first eviction
                    nc.sync.dma_start(beff_sb[:], beff[:, :])

                for h in range(pack // G):
                    # mmA group: 2*G tiles of (L@X)^T into one PSUM bank
                    zp = zps.tile([P, 2 * G * P], f32)
                    for jj in range(G):
                        j = G * h + jj
                        for k in range(2):
                            m = 2 * jj + k
                            nc.tensor.matmul(
                                zp[:, m * P:(m + 1) * P],
                                xb[:, j * E + k * P: j * E + (k + 1) * P],
                                lt_sb,
                                start=True, stop=True,
                            )
                    zt = zsbp.tile([P, 2 * G * P], b_dt)
                    nc.vector.tensor_copy(zt[:], zp[:])
                    # mmB group: G projected j-tiles into one PSUM bank
                    op = ops.tile([P, G * E], f32)
                    for jj in range(G):
                        nc.tensor.matmul(
                            op[:, jj * E:(jj + 1) * E],
                            zt[:, 2 * jj * P:(2 * jj + 1) * P],
                            wt_sb[:, 0:E],
                            start=True, stop=False)
                        nc.tensor.matmul(
                            op[:, jj * E:(jj + 1) * E],
                            zt[:, (2 * jj + 1) * P:(2 * jj + 2) * P],
                            wt_sb[:, E:2 * E],
                            start=False, stop=True)
                    # eviction with fused per-partition bias add
                    ot = yout.tile([P, G * E], o_dt)
                    nc.scalar.activation(ot[:], op[:], ident, bias=beff_sb[:])
                    ysl = y[(tile0 + h * G) * P:(tile0 + (h + 1) * G) * P,
                            :].rearrange("(i p) e -> p i e", p=P)
                    # stores ride SWDGE so input prefetch never queues
                    # behind them on the HWDGE FIFO; the last chunks go
                    # back on HWDGE (inputs are done) to shorten the
                    # SWDGE drain in the kernel tail
                    if ci >= len(SCHED) - 4:
                        nc.sync.dma_start(ysl, ot[:])
                    else:
                        nc.gpsimd.dma_start(ysl, ot[:])
                tile0 += pack
    nc.compile()
    return nc


def _np_dt(name):
    import ml_dtypes
    return {"bfloat16": ml_dtypes.bfloat16}.get(name, np.float32)


def _host_consts(W: np.ndarray, b: np.ndarray, mode: str):
    if mode in ("f32", "f32r"):
        a_np, b_np = np.float32, np.float32
    elif mode == "bf16":
        a_np, b_np = _np_dt("bfloat16"), _np_dt("bfloat16")
    elif mode == "f16":
        a_np, b_np = np.float16, np.float16
    else:
        raise ValueError(mode)

    idx = np.arange(P)
    blk = idx // BS
    LT = ((blk[:, None] == blk[None, :]) & (idx[:, None] < idx[None, :]))
    LT = LT.astype(a_np)

    WT = W.T.astype(np.float32)                      # [E, T]
    WTP = np.concatenate([WT[0:P, :], WT[P:2 * P, :]], axis=1)  # [128, 512]
    WTP = WTP.astype(b_np)
    CW = np.ascontiguousarray(np.concatenate([LT, WTP], axis=1))

    be = b.astype(np.float64).copy()
    be[0] += be[0]
    BEFF = np.ascontiguousarray(
        be[idx % BS].astype(np.float32).reshape(P, 1))
    return CW, BEFF, a_np


def kernel(seq_vector, W, bias):
    mode = MODE
    x = np.asarray(seq_vector, dtype=np.float32)
    W = np.asarray(W, dtype=np.float32)
    b = np.asarray(bias, dtype=np.float32)
    assert x.shape == (N, S, E)

    CW, BEFF, a_np = _host_consts(W, b, mode)
    xs = np.ascontiguousarray(x.astype(a_np))

    if mode not in _cache:
        _cache[mode] = _build_nc(mode)
    nc = _cache[mode]

    in_maps = [
        {"x": xs[i], "cw": CW, "beff": BEFF}
        for i in range(NCORES)
    ]
    res = run_bass_kernel_spmd(nc, in_maps, core_ids=list(range(NCORES)))
    out = np.stack(
        [np.asarray(r["y"], dtype=np.float32) for r in res.results], axis=0)
    return out.reshape(N, S, E)


# revision 28
# speedup vs baseline: 1.0940x; 1.0047x over previous
"""Trainium2 Bass kernel for nn_BlockConvolutionLean.

Computation (see reference):
  features = einsum('nse,te->nst', seq_vector, W)        # 1x1 conv
  blocks of BS=8 along S; out = exclusive-cumsum within block + b_eff
  b_eff = bias with bias[0] doubled at position 0.

Key identity: per 8-token block, Out = L @ X @ W^T + b_eff where L is the
strictly-lower-triangular 8x8 ones matrix.  Both factors are matmuls:

  mmA (cumsum fused into transpose):  Z[e, s'] = sum_s X[s, e] * LT[s, s']
      where LT[s, s'] = 1 iff s, s' in same 8-block and s < s'
      (LT is the 128x128 block-diagonal strictly-upper ones matrix).
      lhsT = X chunk [s(128 part), e(128)] -- natural DMA layout!
      rhs  = LT [s(128 part), s'(128)]
      => Z = (L@X)^T lands e-on-partitions, exactly what mmB needs.

  mmB (projection): O[s', t] = sum_e Z[e, s'] * WT[e, t], K=E=256 via two
      accumulating matmuls.  O is token-on-partition => contiguous store.

  bias: per-partition bias add fused into the PSUM->SBUF eviction
      (ScalarE activation Identity with bias AP [128,1], b_eff[p % 8]).

Sharding: data-parallel over N=8 batches, one batch per NeuronCore.

Precision modes (BCONV_MODE):
  f32   : everything float32 (PE-bound: fp32 matmul = 4 cyc/row)
  f32r  : float32r matmuls (single-pass fp32; LDWEIGHTS-bound)
  bf16  : host stages x as bf16 (halves input HBM traffic); all matmul
          operands bf16, fp32 PSUM accumulation, f32 output
  f16   : like bf16 but float16 operands (10-bit mantissa) AND the
          output staged as f16 (host upcasts) -- halves store traffic
"""

import os

import numpy as np

import concourse.mybir as mybir
import concourse.tile as tile
from concourse import bacc
from concourse.bass_utils import run_bass_kernel_spmd

N, S, E, BS = 8, 8192, 256, 8
P = 128                 # tokens per tile / partitions
NCORES = 8
G = 2                   # j-tiles per PSUM group (one 2KB bank each)
# chunk schedule in 128-token tiles: small chunks at both ends cut the
# time-to-first-matmul and the last-store tail; 8-tile steady state
SCHED = [2, 2, 4] + [8] * 6 + [4, 2, 2]
assert sum(SCHED) == S // P and all(c % G == 0 for c in SCHED)

MODE = os.environ.get("BCONV_MODE", "f16")

_cache = {}


def _build_nc(mode: str):
    f32 = mybir.dt.float32
    f32r = mybir.dt.float32r
    bf16 = mybir.dt.bfloat16
    f16 = mybir.dt.float16

    # a_dt: mmA operands (x, lt); b_dt: mmB operands (z, wt);
    # o_dt: output staging dtype
    if mode == "f32":
        a_dt, b_dt, o_dt = f32, f32, f32
    elif mode == "f32r":
        a_dt, b_dt, o_dt = f32r, f32r, f32
    elif mode == "bf16":
        a_dt, b_dt, o_dt = bf16, bf16, f32
    elif mode == "f16":
        a_dt, b_dt, o_dt = f16, f16, f16
    else:
        raise ValueError(mode)

    nc = bacc.Bacc(
        "TRN2", target_bir_lowering=False, debug=False, num_devices=NCORES)
    # x arrives pre-staged in the matmul dtype (host casts for bf16;
    # f32r is bit-identical to f32) -> every load is a plain HWDGE DMA.
    x = nc.dram_tensor("x", [S, E], a_dt, kind="ExternalInput")
    # lt and wt fused into one tensor -> one DMA, one completion wait
    cw = nc.dram_tensor("cw", [P, P + 2 * E], a_dt, kind="ExternalInput")
    beff = nc.dram_tensor("beff", [P, 1], f32, kind="ExternalInput")
    y = nc.dram_tensor("y", [S, E], o_dt, kind="ExternalOutput")

    ident = mybir.ActivationFunctionType.Identity
    max_pack = max(SCHED)

    with tile.TileContext(nc) as tc:
        with (
            tc.tile_pool(name="const", bufs=1) as constp,
            tc.tile_pool(name="xin", bufs=6) as xin,
            tc.tile_pool(name="zsb", bufs=6) as zsbp,
            tc.tile_pool(name="yout", bufs=8) as yout,
            tc.tile_pool(name="zps", bufs=3, space="PSUM") as zps,
            tc.tile_pool(name="ops", bufs=3, space="PSUM") as ops,
            tc.tile_pool(name="wps", bufs=1, space="PSUM") as wps,
        ):
            cw_sb = constp.tile([P, P + 2 * E], a_dt)
            nc.sync.dma_start(cw_sb[:], cw[:, :])
            lt_sb = cw_sb[:, 0:P]
            wt_sb = cw_sb[:, P:P + 2 * E]
            beff_sb = constp.tile([P, 1], f32)

            # HAM warmup: keep the PE busy while the first input chunks
            # stream in, so real matmuls start at 2.4 GHz (K=8/8).
            wset = constp.tile([P, P], a_dt)
            nc.gpsimd.memset(wset[:], 0.0)
            wp = wps.tile([P, P], f32)
            for _ in range(28):
                nc.tensor.matmul(wp[:], wset[:], wset[:], start=True, stop=True)

            tile0 = 0
            for ci, pack in enumerate(SCHED):
                chunk = pack * P
                xsl = x[tile0 * P:tile0 * P + chunk, :].rearrange(
                    "(i p) e -> p i e", p=P)
                xb = xin.tile([P, max_pack * E], a_dt, tag="xb")
                nc.sync.dma_start(xb[:, 0:pack * E], xsl)
                if ci == 0:
                    # bias needed only at the first eviction
                    nc.sync.dma_start(beff_sb[:], beff[:, :])

                for h in range(pack // G):
                    # mmA group: 2*G tiles of (L@X)^T into one PSUM bank
                    zp = zps.tile([P, 2 * G * P], f32)
                    for jj in range(G):
                        j = G * h + jj
                        for k in range(2):
                            m = 2 * jj + k
                            nc.tensor.matmul(
                                zp[:, m * P:(m + 1) * P],
                                xb[:, j * E + k * P: j * E + (k + 1) * P],
                                lt_sb,
                                start=True, stop=True,
                            )
                    zt = zsbp.tile([P, 2 * G * P], b_dt)
                    nc.vector.tensor_copy(zt[:], zp[:])
                    # mmB group: G projected j-tiles into one PSUM bank
                    op = ops.tile([P, G * E], f32)
                    for jj in range(G):
                        nc.tensor.matmul(
                            op[:, jj * E:(jj + 1) * E],
                            zt[:, 2 * jj * P:(2 * jj + 1) * P],
                            wt_sb[:, 0:E],
                            start=True, stop=False)
                        nc.tensor.matmul(
                            op[:, jj * E:(jj + 1) * E],
                            zt[:, (2 * jj + 1) * P:(2 * jj + 2) * P],
                            wt_sb[:, E:2 * E],
                            start=False, stop=True)
                    # eviction with fused per-partition bias add
                    ot = yout.tile([P, G * E], o_dt)
                    nc.scalar.activation(ot[:], op[:], ident, bias=beff_sb[:])
                    ysl = y[(tile0 + h * G) * P:(tile0 + (h + 1) * G) * P,
                            :].rearrange("(i p) e -> p i e", p=P)
                    # stores ride SWDGE so input prefetch never queues
                    # behind them on the HWDGE FIFO; the last chunks go
                    # back on HWDGE (inputs are done) to shorten the
                    # SWDGE drain in the kernel tail
                    if ci >= len(SCHED) - 4:
                        nc.sync.dma_start(ysl, ot[:])
                    else:
                        nc.gpsimd.dma_start(ysl, ot[:])
                tile0 += pack
    nc.compile()
    return nc


def _np_dt(name):
    import ml_dtypes
    return {"bfloat16": ml_dtypes.bfloat16}.get(name, np.float32)


def _host_consts(W: np.ndarray, b: np.ndarray, mode: str):
    if mode in ("f32", "f32r"):
        a_np, b_np = np.float32, np.float32
    elif mode == "bf16":
        a_np, b_np = _np_dt("bfloat16"), _np_dt("bfloat16")
    elif mode == "f16":
        a_np, b_np = np.float16, np.float16
    else:
        raise ValueError(mode)

    idx = np.arange(P)
    blk = idx // BS
    LT = ((blk[:, None] == blk[None, :]) & (idx[:, None] < idx[None, :]))
    LT = LT.astype(a_np)

    WT = W.T.astype(np.float32)                      # [E, T]
    WTP = np.concatenate([WT[0:P, :], WT[P:2 * P, :]], axis=1)  # [128, 512]
    WTP = WTP.astype(b_np)
    CW = np.ascontiguousarray(np.concatenate([LT, WTP], axis=1))

    be = b.astype(np.float64).copy()
    be[0] += be[0]
    BEFF = np.ascontiguousarray(
        be[idx % BS].astype(np.float32).reshape(P, 1))
    return CW, BEFF, a_np


def kernel(seq_vector, W, bias):
    mode = MODE
    x = np.asarray(seq_vector, dtype=np.float32)
    W = np.asarray(W, dtype=np.float32)
    b = np.asarray(bias, dtype=np.float32)
    assert x.shape == (N, S, E)

    CW, BEFF, a_np = _host_consts(W, b, mode)
    xs = np.ascontiguousarray(x.astype(a_np))

    if mode not in _cache:
        _cache[mode] = _build_nc(mode)
    nc = _cache[mode]

    in_maps = [
        {"x": xs[i], "cw": CW, "beff": BEFF}
        for i in range(NCORES)
    ]
    res = run_bass_kernel_spmd(nc, in_maps, core_ids=list(range(NCORES)))
    out = np.stack(
        [np.asarray(r["y"], dtype=np.float32) for r in res.results], axis=0)
    return out.reshape(N, S, E)
